# revision 20
# baseline (speedup 1.0000x reference)
"""EnsembleFraudDetector GNN (GraphSAGE + GAT + TransformerConv) on 8 trn2 cores.

Sharding: nodes partitioned into 8 dst-blocks. Each core owns the edges into its
block (sorted by dst). Per 128-node block, host-built one-hot matrices M
[edge,dst] / MT [dst,edge] turn segment-sums and q[dst]-expansion into PE
matmuls accumulated in PSUM. Per-edge source features are row-gathered with
dma_gather (int16 indices) from per-core compact halo tables; stage-B/C halo
tables are exchanged with AllToAll of exactly the rows each core references.
Segment softmax is computed without max-subtraction (scores are O(1)) as an
unnormalized weighted aggregation with the denominator as extra matmul columns.
"""
import math
import numpy as np

import concourse.bass as bass
import concourse.bacc as bacc
import concourse.tile as tile
from concourse import mybir
from concourse import bass_utils

P = 128
NCORES = 8
F32 = mybir.dt.float32
BF16 = mybir.dt.bfloat16
I16 = mybir.dt.int16
AF = mybir.ActivationFunctionType
ALU = mybir.AluOpType


def _wrap_idx(idx, pad_to=None):
    """int16 index array -> dma_gather layout [128, ceil(n/16)] (16-partition
    wrap replicated across the 8 gpsimd cores)."""
    n = len(idx)
    if pad_to is None:
        pad_to = ((n + 15) // 16) * 16
    buf = np.zeros(pad_to, np.int16)
    buf[:n] = idx
    w = buf.reshape(-1, 16).T  # [16, pad/16]
    return np.tile(w, (8, 1)).astype(np.int16)


class Plan:
    pass


def make_plan(src, dst, n, nb):
    """Host-side edge partitioning. src/dst int64 [E]. Returns Plan with
    per-core data and shared static structure."""
    pl = Plan()
    ncores = NCORES
    nbp = ((nb + P - 1) // P) * P      # padded own rows
    nblk = nbp // P
    pl.nb, pl.nbp, pl.nblk = nb, nbp, nblk

    deg = np.bincount(dst, minlength=n).astype(np.float32)
    deg_inv = 1.0 / np.maximum(deg, 1.0)

    cores = []
    for c in range(ncores):
        n0 = c * nb
        sel = (dst >= n0) & (dst < n0 + nb)
        es = src[sel].astype(np.int64)
        ed = (dst[sel] - n0).astype(np.int64)
        o = np.argsort(ed, kind="stable")
        es, ed = es[o], ed[o]
        u, upos = np.unique(es, return_inverse=True)
        d = {"es": es, "ed": ed, "u": u, "upos": upos,
             "deg_inv": deg_inv[n0:n0 + nb]}
        cores.append(d)

    # Pmax: max rows any (src-core -> dst-core) pair exchanges, padded to 128.
    pmax = 1
    for c in range(ncores):
        cnt = np.bincount(cores[c]["u"] // nb, minlength=ncores)
        pmax = max(pmax, int(cnt.max()))
    pmax = ((pmax + P - 1) // P) * P
    u8 = pmax * ncores
    assert u8 <= 32767, f"halo table too big for int16: {u8}"
    pl.pmax, pl.u8 = pmax, u8

    # halo row of u[i]: chunk (u//nb)*pmax + rank within chunk
    for c in range(ncores):
        d = cores[c]
        u = d["u"]
        blkid = u // nb
        starts = np.searchsorted(blkid, np.arange(ncores))
        hrow = np.empty(len(u), np.int64)
        for s in range(ncores):
            e = starts[s + 1] if s + 1 < ncores else len(u)
            hrow[starts[s]:e] = s * pmax + np.arange(e - starts[s])
        d["hrow"] = hrow            # halo-table row of each unique src
        d["gsrc"] = hrow[d["upos"]]  # per-edge halo row

    # contribution index list for each SENDER core c: for each dest core dd,
    # the local own-rows (within c's block) that dd references, padded to pmax.
    for c in range(ncores):
        cid = np.zeros(u8, np.int64)
        for dd in range(ncores):
            ud = cores[dd]["u"]
            rows = ud[(ud >= c * nb) & (ud < (c + 1) * nb)] - c * nb
            cid[dd * pmax:dd * pmax + len(rows)] = rows
        cores[c]["cidx"] = cid

    # block/tile structure (shared): T_b = max over cores ceil(edges_in_b/128)
    tb = []
    ranges = []  # per core: per block (lo, hi)
    for c in range(ncores):
        ed = cores[c]["ed"]
        lo = np.searchsorted(ed, np.arange(nblk) * P)
        hi = np.searchsorted(ed, (np.arange(nblk) + 1) * P)
        ranges.append((lo, hi))
    for b in range(nblk):
        t = 1
        for c in range(ncores):
            lo, hi = ranges[c]
            t = max(t, -(-(int(hi[b] - lo[b])) // P))
        tb.append(t)
    pl.tb = tb
    nt = sum(tb)
    pl.nt = nt

    # per-core per-tile: gather idx (into halo tables) + M/MT one-hots
    for c in range(ncores):
        d = cores[c]
        lo, hi = ranges[c]
        gsrc = d["gsrc"]
        ed = d["ed"]
        gidx = np.zeros((nt, P), np.int64)
        Ms = np.zeros((nt, P, P), np.float32)
        t = 0
        for b in range(nblk):
            l, h = int(lo[b]), int(hi[b])
            for j in range(tb[b]):
                e0 = l + j * P
                e1 = min(l + (j + 1) * P, h)
                if e1 > e0:
                    k = e1 - e0
                    gidx[t, :k] = gsrc[e0:e1]
                    Ms[t, np.arange(k), ed[e0:e1] - b * P] = 1.0
                t += 1
        d["gidx"] = gidx
        d["M"] = Ms
    return pl, cores


def _bf(a):
    import ml_dtypes
    return np.asarray(a).astype(ml_dtypes.bfloat16)


def _np(a):
    return np.asarray(a, np.float32)


def build_inputs(x, params, pl, cores, gchunk_a, gchunk_b):
    """Per-core input dicts (all same shapes)."""
    N, IN = x.shape
    HID, H = 64, 8
    HD = H * HID
    nb, nbp, u8 = pl.nb, pl.nbp, pl.u8
    p = params
    x = _np(x)

    # combined attention vectors
    W1 = _np(p["gat1_W"]).reshape(IN, H, HID)
    w_as1 = np.einsum("ihc,hc->ih", W1, _np(p["gat1_asrc"]))
    w_ad1 = np.einsum("ihc,hc->ih", W1, _np(p["gat1_adst"]))
    W2g = _np(p["gat2_W"])  # [512, 64]
    w_as2 = W2g @ _np(p["gat2_asrc"])[0]
    w_ad2 = W2g @ _np(p["gat2_adst"])[0]

    # halo rhs [166, 1664]: z1 |k1 |v1 |p1 |a_s1 |pad  (ones row adds biases)
    wh = np.zeros((IN + 1, 1664), np.float32)
    wh[:IN, 0:512] = _np(p["gat1_W"])
    wh[:IN, 512:1024] = _np(p["t1_Wk"]); wh[IN, 512:1024] = _np(p["t1_bk"])
    wh[:IN, 1024:1536] = _np(p["t1_Wv"]); wh[IN, 1024:1536] = _np(p["t1_bv"])
    wh[:IN, 1536:1600] = _np(p["sage_Wl"][0])
    wh[:IN, 1600:1608] = w_as1

    # own rhs [166, 1616]: q1*0.125 |skip1 |z1o |sage_r |a_s1o |a_d1o
    wo = np.zeros((IN + 1, 1616), np.float32)
    wo[:IN, 0:512] = _np(p["t1_Wq"]) * 0.125
    wo[IN, 0:512] = _np(p["t1_bq"]) * 0.125
    wo[:IN, 512:1024] = _np(p["t1_Wskip"]); wo[IN, 512:1024] = _np(p["t1_bskip"])
    wo[:IN, 1024:1536] = _np(p["gat1_W"])
    wo[:IN, 1536:1600] = _np(p["sage_Wr"][0])
    wo[:IN, 1600:1608] = w_as1
    wo[:IN, 1608:1616] = w_ad1

    w_gat2 = np.zeros((HD, 66), np.float32)
    w_gat2[:, 0:64] = W2g; w_gat2[:, 64] = w_as2; w_gat2[:, 65] = w_ad2

    w_t2 = np.zeros((HD, 256), np.float32)
    w_t2[:, 0:64] = _np(p["t2_Wq"]) * 0.125
    w_t2[:, 64:128] = _np(p["t2_Wk"])
    w_t2[:, 128:192] = _np(p["t2_Wv"])
    w_t2[:, 192:256] = _np(p["t2_Wskip"])
    t2_bias = np.concatenate([_np(p["t2_bq"]) * 0.125, _np(p["t2_bk"]),
                              _np(p["t2_bv"]), _np(p["t2_bskip"])])[None, :]

    w_sage2 = np.concatenate([_np(p["sage_Wl"][1]), _np(p["sage_Wr"][1])], 1)
    w_sage3 = np.concatenate([_np(p["sage_Wl"][2]), _np(p["sage_Wr"][2])], 1)
    w_cls = np.concatenate([_np(p["gat_cls_W"]), _np(p["t_cls_W"])], 1)

    # bias tiles [128, 710]
    bt = np.zeros((P, 710), np.float32)
    bt[:, 0:512] = _np(p["gat1_b"])[None]
    bt[:, 512:576] = _np(p["sage_bl"][0])[None]
    bt[:, 576:640] = _np(p["sage_bl"][1])[None]
    bt[:, 640:704] = _np(p["gat2_b"])[None]
    bt[:, 704:706] = _np(p["sage_bl"][2])[None]
    bt[:, 706:708] = _np(p["gat_cls_b"])[None]
    bt[:, 708:710] = _np(p["t_cls_b"])[None]

    shared = {
        "w_halo": _bf(wh), "w_own": _bf(wo), "w_gat2": _bf(w_gat2),
        "w_t2": _bf(w_t2), "t2_bias": _bf(t2_bias), "w_sage2": _bf(w_sage2),
        "w_sage3": _bf(w_sage3), "w_cls": _bf(w_cls), "biases": _bf(bt),
    }

    maps = []
    for c in range(NCORES):
        d = cores[c]
        xo = np.zeros((nbp, IN + 1), np.float32)
        xo[:nb, :IN] = x[c * nb:(c + 1) * nb]
        xo[:nb, IN] = 1.0
        xh = np.zeros((u8, IN + 1), np.float32)
        xh[d["hrow"], :IN] = x[d["u"]]
        xh[d["hrow"], IN] = 1.0

        # 16-wrap commutes with concatenation at multiples of 16, so one
        # global wrap serves every chunking.
        gidx_w = _wrap_idx(d["gidx"].reshape(-1))
        cidx_w = _wrap_idx(d["cidx"])

        dinv = np.ones((nbp, 1), np.float32)
        dinv[:nb, 0] = d["deg_inv"]

        m = {
            "xoT": _bf(xo.T.copy()), "xhT": _bf(xh.T.copy()),
            "gidx": gidx_w, "cidx": cidx_w,
            "Mb": _bf(d["M"].reshape(pl.nt * P, P)),
            "MTb": _bf(np.transpose(d["M"], (0, 2, 1)).reshape(pl.nt * P, P).copy()),
            "deginv": dinv,
        }
        m.update(shared)
        maps.append(m)
    return maps


def build_program(pl, wts, gchunk_a, gchunk_b, n_in=165):
    """Construct the Bass/Tile program (same for all cores)."""
    nbp, nblk, nt, u8 = pl.nbp, pl.nblk, pl.nt, pl.u8
    tb = pl.tb
    INP = n_in + 1
    nc = bacc.Bacc("TRN2", target_bir_lowering=False, debug=False,
                   num_devices=NCORES)

    def din(name, shape, dt):
        return nc.dram_tensor(name, shape, dt, kind="ExternalInput")

    xoT = din("xoT", [INP, nbp], BF16)
    xhT = din("xhT", [INP, u8], BF16)
    gidx = din("gidx", [P, (nt * P) // 16], I16)
    cidx = din("cidx", [P, u8 // 16], I16)
    Mb = din("Mb", [nt * P, P], BF16)
    MTb = din("MTb", [nt * P, P], BF16)
    deginv = din("deginv", [nbp, 1], F32)
    w_halo = din("w_halo", [INP, 1664], BF16)
    w_own = din("w_own", [INP, 1616], BF16)
    w_gat2 = din("w_gat2", [512, 66], BF16)
    w_t2 = din("w_t2", [512, 256], BF16)
    t2_bias = din("t2_bias", [1, 256], BF16)
    w_sage2 = din("w_sage2", [64, 128], BF16)
    w_sage3 = din("w_sage3", [64, 4], BF16)
    w_cls = din("w_cls", [64, 4], BF16)
    biases = din("biases", [P, 710], BF16)
    out = nc.dram_tensor("out", [pl.nb, 2], F32, kind="ExternalOutput")
    if DEBUG:
        dbg1 = nc.dram_tensor("dbg1", [P, 1664], BF16, kind="ExternalOutput")
        dbg2 = nc.dram_tensor("dbg2", [P, 520], BF16, kind="ExternalOutput")
        dbg3 = nc.dram_tensor("dbg3", [P, 384], BF16, kind="ExternalOutput")
        dbg4 = nc.dram_tensor("dbg4", [P, 1664], F32, kind="ExternalOutput")

    from contextlib import ExitStack
    with tile.TileContext(nc) as tc, ExitStack() as ctx:
        _emit(ctx, tc, nc, pl, wts, gchunk_a, gchunk_b, locals())
    nc.compile()
    return nc


def _emit(ctx, tc, nc, pl, wts, GA, GB, T):
    nbp, nblk, nt, u8, nb = pl.nbp, pl.nblk, pl.nt, pl.u8, pl.nb
    tb = pl.tb
    INP = T["xoT"].shape[0]

    dram = ctx.enter_context(tc.tile_pool(name="dram", bufs=1, space="DRAM"))
    sb = ctx.enter_context(tc.tile_pool(name="sb", bufs=1))
    sb2 = ctx.enter_context(tc.tile_pool(name="sb2", bufs=2))
    ps = ctx.enter_context(tc.tile_pool(name="ps", bufs=1, space="PSUM"))
    ps2 = ctx.enter_context(tc.tile_pool(name="ps2", bufs=2, space="PSUM"))

    # ---- internal DRAM tables ----
    zkvpa = dram.tile([u8, 1664], BF16, name="zkvpa")
    q1a = dram.tile([nbp, 520], BF16, name="q1a")
    z1o = dram.tile([nbp, 512], BF16, name="z1o")
    skip1 = dram.tile([nbp, 512], BF16, name="skip1")
    aux1 = dram.tile([nbp, 80], BF16, name="aux1")   # sager1 |a_s1o |a_d1o
    packB = dram.tile([nbp, 384], BF16, name="packB")
    q2a = dram.tile([nbp, 65], BF16, name="q2a")
    aux2 = dram.tile([nbp, 128], BF16, name="aux2")  # skip2 | r2
    contribB = dram.tile([u8, 384], BF16, name="contribB")
    packBH = dram.tile([u8, 384], BF16, name="packBH")
    p3t = dram.tile([nbp, 64], F32, name="p3t")
    gto = dram.tile([nbp, 8], F32, name="gto")       # gat_out|trans_out|r3|pad
    contribC = dram.tile([u8, 64], F32, name="contribC")
    p3H = dram.tile([u8, 64], F32, name="p3H")

    # ---- pinned SBUF ----
    wh_a = sb.tile([P, 1664], BF16, name="wh_a")
    wh_b = sb.tile([INP - P, 1664], BF16, name="wh_b")
    nc.sync.dma_start(out=wh_a[:], in_=T["w_halo"][0:P, :])
    nc.sync.dma_start(out=wh_b[:], in_=T["w_halo"][P:INP, :])
    wo_a = sb.tile([P, 1616], BF16, name="wo_a")
    wo_b = sb.tile([INP - P, 1616], BF16, name="wo_b")
    nc.sync.dma_start(out=wo_a[:], in_=T["w_own"][0:P, :])
    nc.sync.dma_start(out=wo_b[:], in_=T["w_own"][P:INP, :])
    wg2 = sb.tile([P, 4, 66], BF16, name="wg2")
    nc.sync.dma_start(out=wg2[:], in_=T["w_gat2"].ap().rearrange("(c p) n -> p c n", p=P))
    wt2 = sb.tile([P, 4, 256], BF16, name="wt2")
    nc.sync.dma_start(out=wt2[:], in_=T["w_t2"].ap().rearrange("(c p) n -> p c n", p=P))
    t2b = sb.tile([1, 256], BF16, name="t2b")
    nc.sync.dma_start(out=t2b[:], in_=T["t2_bias"][:, :])
    ws2 = sb.tile([64, 128], BF16, name="ws2")
    nc.sync.dma_start(out=ws2[:], in_=T["w_sage2"][:, :])
    ws3 = sb.tile([64, 4], BF16, name="ws3")
    nc.sync.dma_start(out=ws3[:], in_=T["w_sage3"][:, :])
    wcl = sb.tile([64, 4], BF16, name="wcl")
    nc.sync.dma_start(out=wcl[:], in_=T["w_cls"][:, :])
    bia = sb.tile([P, 710], BF16, name="bia")
    nc.sync.dma_start(out=bia[:], in_=T["biases"][:, :])
    gidx_sb = sb.tile([P, (nt * P) // 16], I16, name="gidx_sb")
    nc.sync.dma_start(out=gidx_sb[:], in_=T["gidx"][:, :])
    cidx_sb = sb.tile([P, u8 // 16], I16, name="cidx_sb")
    nc.sync.dma_start(out=cidx_sb[:], in_=T["cidx"][:, :])
    ones_r = sb.tile([1, P], BF16, name="ones_r")
    nc.vector.memset(ones_r[:], 1.0)
    ident = sb.tile([P, P], BF16, name="ident")
    from concourse.masks import make_identity
    make_identity(nc, ident[:])

    def bslice(c0, c1):
        return bia[:, c0:c1]

    # =========== phase 0: projections ===========
    def proj_pass(xT, rows, w_a, w_b, ncols, groups, evac, tagp):
        """rows: number of row-tiles; groups: list of (c0,c1); evac(r, psums)"""
        XCH = 8
        for r0 in range(0, rows, XCH):
            r1 = min(r0 + XCH, rows)
            xa = sb2.tile([P, XCH * P], BF16, tag="p0xa", name="xa")
            xb = sb2.tile([INP - P, XCH * P], BF16, tag="p0xb", name="xb")
            nc.sync.dma_start(out=xa[:, 0:(r1 - r0) * P],
                              in_=xT[0:P, r0 * P:r1 * P])
            nc.sync.dma_start(out=xb[:, 0:(r1 - r0) * P],
                              in_=xT[P:INP, r0 * P:r1 * P])
            for r in range(r0, r1):
                sl = slice((r - r0) * P, (r - r0 + 1) * P)
                psl = []
                for gi, (c0, c1) in enumerate(groups):
                    pt = ps2.tile([P, c1 - c0], F32,
                                  tag=("psA" if gi % 2 == 0 else "psB"),
                                  name="pt", space="PSUM")
                    nc.tensor.matmul(pt[:], lhsT=xa[:, sl], rhs=w_a[:, c0:c1],
                                     start=True, stop=False)
                    nc.tensor.matmul(pt[:], lhsT=xb[:, sl], rhs=w_b[:, c0:c1],
                                     start=False, stop=True)
                    psl.append(pt)
                evac(r, psl)

    GRP_H = [(0, 512), (512, 1024), (1024, 1536), (1536, 1608)]

    def evac_halo(r, psl):
        ev = sb2.tile([P, 1664], BF16, tag="ev_h", name="ev")
        nc.scalar.activation(ev[:, 0:512], psl[0][:], AF.Copy)
        nc.scalar.activation(ev[:, 512:1024], psl[1][:], AF.Copy)
        nc.vector.tensor_copy(out=ev[:, 1024:1536], in_=psl[2][:])
        nc.vector.tensor_copy(out=ev[:, 1536:1608], in_=psl[3][:])
        nc.sync.dma_start(out=zkvpa[r * P:(r + 1) * P, :], in_=ev[:])

    proj_pass(T["xhT"], u8 // P, wh_a, wh_b, 1664, GRP_H, evac_halo, "h")

    GRP_O = [(0, 512), (512, 1024), (1024, 1536), (1536, 1616)]

    def evac_own(r, psl):
        rs = slice(r * P, (r + 1) * P)
        ev = sb2.tile([P, 1616], BF16, tag="ev_o", name="ev2")
        nc.scalar.activation(ev[:, 0:512], psl[0][:], AF.Copy)      # q1s
        nc.scalar.activation(ev[:, 512:1024], psl[1][:], AF.Copy)   # skip1
        nc.vector.tensor_copy(out=ev[:, 1024:1536], in_=psl[2][:])  # z1o
        nc.vector.tensor_copy(out=ev[:, 1536:1616], in_=psl[3][:])  # sr|as|ad
        qa = sb2.tile([P, 520], BF16, tag="ev_qa", name="qa")
        nc.vector.tensor_copy(out=qa[:, 0:512], in_=ev[:, 0:512])
        nc.vector.tensor_copy(out=qa[:, 512:520], in_=ev[:, 1608:1616])
        nc.sync.dma_start(out=q1a[rs, :], in_=qa[:])
        nc.sync.dma_start(out=skip1[rs, :], in_=ev[:, 512:1024])
        nc.sync.dma_start(out=z1o[rs, :], in_=ev[:, 1024:1536])
        nc.sync.dma_start(out=aux1[rs, :], in_=ev[:, 1536:1616])

    proj_pass(T["xoT"], nblk, wo_a, wo_b, 1616, GRP_O, evac_own, "o")

    # =========== helper: gather chunk machinery ===========
    def chunk_loader(table, width, gname, chunk_tiles, idx_sb, nt_total,
                     dt=BF16, need_mt=True):
        state = {"cur": -1, "g": None, "m": None, "mt": None}

        def get(t):
            ch = t // chunk_tiles
            if ch != state["cur"]:
                state["cur"] = ch
                t0 = ch * chunk_tiles
                t1 = min(t0 + chunk_tiles, nt_total)
                k = t1 - t0
                g = sb2.tile([P, chunk_tiles, width], dt, tag=f"g_{gname}",
                             name="gch")
                nc.gpsimd.dma_gather(
                    out_ap=g[:, 0:k, :],
                    in_ap=table[:, :],
                    idxs_ap=idx_sb[:, t0 * 8:t0 * 8 + ((k * P) // 16)],
                    num_idxs=k * P, num_idxs_reg=k * P, elem_size=width)
                m = sb2.tile([P, chunk_tiles * P], BF16, tag=f"m_{gname}",
                             name="mch")
                nc.sync.dma_start(
                    out=m[:, 0:k * P],
                    in_=Mb_ap[:, t0:t1, :])
                state["g"], state["m"] = g, m
                if need_mt:
                    mt = sb2.tile([P, chunk_tiles * P], BF16,
                                  tag=f"mt_{gname}", name="mtch")
                    nc.sync.dma_start(
                        out=mt[:, 0:k * P],
                        in_=MTb_ap[:, t0:t1, :])
                    state["mt"] = mt
            i = t - state["cur"] * chunk_tiles
            mt = state["mt"]
            return (state["g"][:, i, :], state["m"][:, i * P:(i + 1) * P],
                    mt[:, i * P:(i + 1) * P] if mt is not None else None)
        return get

    Mb_ap = T["Mb"].ap().rearrange("(t e) d -> e t d", e=P)
    MTb_ap = T["MTb"].ap().rearrange("(t d) e -> d t e", d=P)

    # =========== pass A: edge loop 1 + block post ===========
    getA = chunk_loader(zkvpa, 1664, "A", GA, gidx_sb, nt)
    t_global = 0
    for b in range(nblk):
        rs = slice(b * P, (b + 1) * P)
        qa_b = sb2.tile([P, 520], BF16, tag="qa_b", name="qab")
        nc.sync.dma_start(out=qa_b[:], in_=q1a[rs, :])
        aggG = ps.tile([P, 512], F32, tag="aggG", name="aggG", space="PSUM")
        aggT = ps.tile([P, 512], F32, tag="aggT", name="aggT", space="PSUM")
        aggS = ps.tile([P, 96], F32, tag="aggS", name="aggS", space="PSUM")
        ntb = tb[b]
        for j in range(ntb):
            g_t, m_t, mt_t = getA(t_global)
            z_t, k_t, v_t = g_t[0:P, 0:512], g_t[0:P, 512:1024], g_t[0:P, 1024:1536]
            p1_t, as1_t = g_t[0:P, 1536:1600], g_t[0:P, 1600:1608]
            st, sp = (j == 0), (j == ntb - 1)
            qexq = ps2.tile([P, 512], F32, tag="psA", name="qexq", space="PSUM")
            nc.tensor.matmul(qexq[:], lhsT=mt_t, rhs=qa_b[:, 0:512],
                             start=True, stop=True)
            qexd = ps2.tile([P, 8], F32, tag="psB", name="qexd", space="PSUM")
            nc.tensor.matmul(qexd[:], lhsT=mt_t, rhs=qa_b[:, 512:520],
                             start=True, stop=True)
            qe = sb2.tile([P, 512], BF16, tag="qe", name="qe")
            nc.scalar.activation(qe[:], qexq[:], AF.Copy)
            ad = sb2.tile([P, 8], BF16, tag="ad", name="ad")
            nc.vector.tensor_copy(out=ad[:], in_=qexd[:])
            qk = sb2.tile([P, 512], BF16, tag="qk", name="qk")
            nc.vector.tensor_tensor(out=qk[:], in0=qe[:], in1=k_t, op=ALU.mult)
            sc = sb2.tile([P, 16], F32, tag="sc", name="sc")
            nc.vector.tensor_reduce(
                out=sc[:, 8:16], in_=qk[:].rearrange("p (h c) -> p h c", h=8),
                axis=mybir.AxisListType.X, op=ALU.add)
            gs = sb2.tile([P, 16], F32, tag="gs", name="gs")
            nc.vector.tensor_tensor(out=gs[:, 0:8], in0=as1_t, in1=ad[:],
                                    op=ALU.add)
            nc.vector.tensor_scalar(out=gs[:, 8:16], in0=gs[:, 0:8],
                                    scalar1=0.2, scalar2=None, op0=ALU.mult)
            nc.vector.tensor_tensor(out=sc[:, 0:8], in0=gs[:, 0:8],
                                    in1=gs[:, 8:16], op=ALU.max)
            srhs = sb2.tile([P, 80], BF16, tag="srhs", name="srhs")
            nc.vector.tensor_copy(out=srhs[:, 0:64], in_=p1_t)
            w_all = srhs[:, 64:80]
            nc.scalar.activation(w_all, sc[:], AF.Exp)
            wgb = srhs[:, 64:72].rearrange("p (h o) -> p h o", o=1).broadcast_to([P, 8, 64])
            wtb = srhs[:, 72:80].rearrange("p (h o) -> p h o", o=1).broadcast_to([P, 8, 64])
            vG = sb2.tile([P, 512], BF16, tag="vG", name="vG")
            nc.vector.tensor_tensor(out=vG[:].rearrange("p (h c) -> p h c", h=8),
                                    in0=z_t.rearrange("p (h c) -> p h c", h=8),
                                    in1=wgb, op=ALU.mult)
            vT = sb2.tile([P, 512], BF16, tag="vT", name="vT")
            nc.vector.tensor_tensor(out=vT[:].rearrange("p (h c) -> p h c", h=8),
                                    in0=v_t.rearrange("p (h c) -> p h c", h=8),
                                    in1=wtb, op=ALU.mult)
            nc.tensor.matmul(aggG[:], lhsT=m_t, rhs=vG[:], start=st, stop=sp)
            nc.tensor.matmul(aggT[:], lhsT=m_t, rhs=vT[:], start=st, stop=sp)
            nc.tensor.matmul(aggS[:, 0:80], lhsT=m_t, rhs=srhs[:], start=st,
                             stop=sp)
            t_global += 1

        # ---- block post A ----
        z1o_b = sb2.tile([P, 512], BF16, tag="z1o_b", name="z1ob")
        nc.sync.dma_start(out=z1o_b[:], in_=z1o[rs, :])
        sk1_b = sb2.tile([P, 512], BF16, tag="sk1_b", name="sk1b")
        nc.sync.dma_start(out=sk1_b[:], in_=skip1[rs, :])
        ax1_b = sb2.tile([P, 80], BF16, tag="ax1_b", name="ax1b")
        nc.sync.dma_start(out=ax1_b[:], in_=aux1[rs, :])
        di_b = sb2.tile([P, 1], F32, tag="di_b", name="dib")
        nc.sync.dma_start(out=di_b[:], in_=T["deginv"][rs, :])

        # self-loop GAT1 weights: exp(lrelu(a_s + a_d))
        sl_in = sb2.tile([P, 24], F32, tag="sl_in", name="slin")
        nc.vector.tensor_tensor(out=sl_in[:, 0:8], in0=ax1_b[:, 64:72],
                                in1=ax1_b[:, 72:80], op=ALU.add)
        nc.vector.tensor_scalar(out=sl_in[:, 8:16], in0=sl_in[:, 0:8],
                                scalar1=0.2, scalar2=None, op0=ALU.mult)
        nc.vector.tensor_tensor(out=sl_in[:, 16:24], in0=sl_in[:, 0:8],
                                in1=sl_in[:, 8:16], op=ALU.max)
        wslf = sb2.tile([P, 8], BF16, tag="wslf", name="wslf")
        nc.scalar.activation(wslf[:], sl_in[:, 16:24], AF.Exp)

        aggG_s = sb2.tile([P, 512], BF16, tag="aggG_s", name="aggGs")
        nc.scalar.activation(aggG_s[:], aggG[:], AF.Copy)
        aggT_s = sb2.tile([P, 512], BF16, tag="aggT_s", name="aggTs")
        nc.scalar.activation(aggT_s[:], aggT[:], AF.Copy)
        aggS_s = sb2.tile([P, 80], BF16, tag="aggS_s", name="aggSs")
        nc.vector.tensor_copy(out=aggS_s[:], in_=aggS[:, 0:80])

        slfn = sb2.tile([P, 512], BF16, tag="slfn", name="slfn")
        wsb = wslf[:].rearrange("p (h o) -> p h o", o=1).broadcast_to([P, 8, 64])
        nc.vector.tensor_tensor(out=slfn[:].rearrange("p (h c) -> p h c", h=8),
                                in0=z1o_b[:].rearrange("p (h c) -> p h c", h=8),
                                in1=wsb, op=ALU.mult)
        numG = sb2.tile([P, 512], BF16, tag="numG", name="numG")
        nc.vector.tensor_tensor(out=numG[:], in0=aggG_s[:], in1=slfn[:],
                                op=ALU.add)
        dn = sb2.tile([P, 32], F32, tag="dn", name="dn")
        nc.vector.tensor_tensor(out=dn[:, 0:8], in0=aggS_s[:, 64:72],
                                in1=wslf[:], op=ALU.add)      # GAT denom
        nc.vector.tensor_scalar(out=dn[:, 8:16], in0=aggS_s[:, 72:80],
                                scalar1=1e-16, scalar2=None, op0=ALU.max)
        nc.vector.reciprocal(out=dn[:, 16:24], in_=dn[:, 0:8])
        nc.vector.reciprocal(out=dn[:, 24:32], in_=dn[:, 8:16])
        dnb = sb2.tile([P, 16], BF16, tag="dnb", name="dnb")
        nc.vector.tensor_copy(out=dnb[:], in_=dn[:, 16:32])

        def elu(dst_tile, x_tile, tmp_tag):
            xm = sb2.tile([P, 512], BF16, tag=tmp_tag, name="xm")
            nc.vector.tensor_scalar(out=xm[:], in0=x_tile, scalar1=0.0,
                                    scalar2=None, op0=ALU.min)
            em = sb2.tile([P, 512], BF16, tag=tmp_tag + "e", name="em")
            nc.scalar.activation(em[:], xm[:], AF.Exp)
            nc.vector.tensor_scalar(out=xm[:], in0=x_tile, scalar1=0.0,
                                    scalar2=None, op0=ALU.max)
            nc.vector.tensor_tensor(out=em[:], in0=em[:], in1=xm[:], op=ALU.add)
            nc.vector.tensor_scalar(out=dst_tile, in0=em[:], scalar1=-1.0,
                                    scalar2=None, op0=ALU.add)

        # g1 = elu(numG/denG + gat1_b)
        g1p = sb2.tile([P, 512], BF16, tag="g1p", name="g1p")
        dgb = dnb[:, 0:8].rearrange("p (h o) -> p h o", o=1).broadcast_to([P, 8, 64])
        nc.vector.tensor_tensor(out=g1p[:].rearrange("p (h c) -> p h c", h=8),
                                in0=numG[:].rearrange("p (h c) -> p h c", h=8),
                                in1=dgb, op=ALU.mult)
        nc.vector.tensor_tensor(out=g1p[:], in0=g1p[:], in1=bslice(0, 512),
                                op=ALU.add)
        g1 = sb2.tile([P, 512], BF16, tag="g1", name="g1")
        elu(g1[:], g1p[:], "elu1")

        # t1 = elu(aggT/denT + skip1)
        t1p = sb2.tile([P, 512], BF16, tag="t1p", name="t1p")
        dtb = dnb[:, 8:16].rearrange("p (h o) -> p h o", o=1).broadcast_to([P, 8, 64])
        nc.vector.tensor_tensor(out=t1p[:].rearrange("p (h c) -> p h c", h=8),
                                in0=aggT_s[:].rearrange("p (h c) -> p h c", h=8),
                                in1=dtb, op=ALU.mult)
        nc.vector.tensor_tensor(out=t1p[:], in0=t1p[:], in1=sk1_b[:], op=ALU.add)
        t1 = sb2.tile([P, 512], BF16, tag="t1", name="t1")
        elu(t1[:], t1p[:], "elu2")

        # h1 = relu(aggS*deginv + bl1 + sage_r1)
        h1 = sb2.tile([P, 64], BF16, tag="h1", name="h1")
        nc.vector.tensor_scalar(out=h1[:], in0=aggS_s[:, 0:64],
                                scalar1=di_b[:, 0:1], scalar2=None, op0=ALU.mult)
        nc.vector.tensor_tensor(out=h1[:], in0=h1[:], in1=ax1_b[:, 0:64],
                                op=ALU.add)
        nc.vector.tensor_tensor(out=h1[:], in0=h1[:], in1=bslice(512, 576),
                                op=ALU.add)
        nc.vector.tensor_scalar(out=h1[:], in0=h1[:], scalar1=0.0,
                                scalar2=None, op0=ALU.max)

        if DEBUG and b == 0:
            dwr = sb2.tile([P, 1664], F32, tag="dwr", name="dwr")
            nc.vector.tensor_copy(out=dwr[:, 0:512], in_=g1[:])
            nc.vector.tensor_copy(out=dwr[:, 512:1024], in_=t1[:])
            nc.vector.tensor_copy(out=dwr[:, 1024:1088], in_=h1[:])
            nc.vector.tensor_copy(out=dwr[:, 1088:1152], in_=aggS_s[:, 0:64])
            nc.vector.tensor_copy(out=dwr[:, 1152:1216], in_=ax1_b[:, 0:64])
            nc.vector.tensor_copy(out=dwr[:, 1216:1217], in_=di_b[:, 0:1])
            nc.vector.tensor_copy(out=dwr[:, 1600:1616], in_=dn[:, 0:16])
            nc.vector.tensor_copy(out=dwr[:, 1616:1632], in_=aggS_s[:, 64:80])
            nc.vector.tensor_copy(out=dwr[:, 1632:1640], in_=wslf[:])
            nc.sync.dma_start(out=T["dbg4"][:, :], in_=dwr[:])

        # transposes: g1T, t1T (4x128 chunks each), h1T
        tp = ps.tile([P, 512], BF16, tag="tp", name="tp", space="PSUM")
        g1T = sb2.tile([P, 4, P], BF16, tag="g1T", name="g1T")
        for k in range(4):
            nc.tensor.transpose(out=tp[:, k * P:(k + 1) * P],
                                in_=g1[:, k * P:(k + 1) * P], identity=ident[:])
        nc.scalar.activation(g1T[:].rearrange("p k q -> p (k q)"), tp[:], AF.Copy)
        tp2 = ps.tile([P, 512], BF16, tag="tp", name="tp2", space="PSUM")
        t1T = sb2.tile([P, 4, P], BF16, tag="t1T", name="t1T")
        for k in range(4):
            nc.tensor.transpose(out=tp2[:, k * P:(k + 1) * P],
                                in_=t1[:, k * P:(k + 1) * P], identity=ident[:])
        nc.scalar.activation(t1T[:].rearrange("p k q -> p (k q)"), tp2[:], AF.Copy)
        tph = ps.tile([64, P], BF16, tag="tp", name="tph", space="PSUM")
        nc.tensor.transpose(out=tph[:], in_=h1[:], identity=ident[:])
        h1T = sb2.tile([64, P], BF16, tag="h1T", name="h1T")
        nc.vector.tensor_copy(out=h1T[:], in_=tph[:])

        # projections: [z2|as2|ad2] (66), [q2|k2|v2|skip2] (256), [p2|r2] (128)
        prj = ps.tile([P, 66 + 256 + 128], F32, tag="tp", name="prj",
                      space="PSUM")
        for k in range(4):
            nc.tensor.matmul(prj[:, 0:66], lhsT=g1T[:, k, :], rhs=wg2[:, k, :],
                             start=(k == 0), stop=(k == 3))
        for k in range(4):
            nc.tensor.matmul(prj[:, 66:322], lhsT=t1T[:, k, :], rhs=wt2[:, k, :],
                             start=(k == 0), stop=False)
        nc.tensor.matmul(prj[:, 66:322], lhsT=ones_r[:], rhs=t2b[:],
                         start=False, stop=True)
        nc.tensor.matmul(prj[:, 322:450], lhsT=h1T[:], rhs=ws2[:],
                         start=True, stop=True)

        pk = sb2.tile([P, 384], BF16, tag="pk", name="pk")
        nc.scalar.activation(pk[:, 0:64], prj[:, 0:64], AF.Copy)      # z2
        nc.scalar.activation(pk[:, 64:192], prj[:, 130:258], AF.Copy)  # k2|v2
        nc.vector.tensor_copy(out=pk[:, 192:256], in_=prj[:, 322:386])  # p2
        nc.vector.tensor_copy(out=pk[:, 256:257], in_=prj[:, 64:65])  # as2
        nc.sync.dma_start(out=packB[rs, :], in_=pk[:])
        q2_t = sb2.tile([P, 65], BF16, tag="q2_t", name="q2t")
        nc.vector.tensor_copy(out=q2_t[:, 0:64], in_=prj[:, 66:130])  # q2s
        nc.vector.tensor_copy(out=q2_t[:, 64:65], in_=prj[:, 65:66])  # ad2
        nc.sync.dma_start(out=q2a[rs, :], in_=q2_t[:])
        ax2 = sb2.tile([P, 128], BF16, tag="ax2", name="ax2")
        nc.vector.tensor_copy(out=ax2[:, 0:64], in_=prj[:, 258:322])  # skip2
        nc.vector.tensor_copy(out=ax2[:, 64:128], in_=prj[:, 386:450])  # r2
        nc.sync.dma_start(out=aux2[rs, :], in_=ax2[:])

    if DEBUG:
        nc.sync.dma_start(out=T["dbg1"][:, :], in_=zkvpa[0:P, :])
        nc.sync.dma_start(out=T["dbg2"][:, :], in_=q1a[0:P, :])
        nc.sync.dma_start(out=T["dbg3"][:, :], in_=packB[0:P, :])

    # =========== exchange B ===========
    for r0 in range(0, u8, 1024):
        r1 = min(r0 + 1024, u8)
        k = r1 - r0
        pg = sb2.tile([P, 8, 384], BF16, tag="pg", name="pg")
        nc.gpsimd.dma_gather(
            out_ap=pg[:, 0:k // P, :], in_ap=packB[:, :],
            idxs_ap=cidx_sb[:, r0 // 16:r0 // 16 + k // 16],
            num_idxs=k, num_idxs_reg=k, elem_size=384)
        nc.sync.dma_start(
            out=contribB[r0:r1, :].rearrange("(t p) e -> p t e", p=P),
            in_=pg[:, 0:k // P, :])
    nc.gpsimd.collective_compute(
        "AllToAll", ALU.bypass, replica_groups=[list(range(NCORES))],
        ins=[contribB[:]], outs=[packBH[:]])

    # =========== pass B: edge loop 2 + block post ===========
    getB = chunk_loader(packBH, 384, "B", GB, gidx_sb, nt)
    t_global = 0
    for b in range(nblk):
        rs = slice(b * P, (b + 1) * P)
        qa2_b = sb2.tile([P, 65], BF16, tag="qa2_b", name="qa2b")
        nc.sync.dma_start(out=qa2_b[:], in_=q2a[rs, :])
        aggB = ps.tile([P, 194], F32, tag="aggG", name="aggB", space="PSUM")
        ntb = tb[b]
        for j in range(ntb):
            g_t, m_t, mt_t = getB(t_global)
            z2_t, k2_t = g_t[0:P, 0:64], g_t[0:P, 64:128]
            v2_t, p2_t, as2_t = g_t[0:P, 128:192], g_t[0:P, 192:256], g_t[0:P, 256:257]
            st, sp = (j == 0), (j == ntb - 1)
            qx2 = ps2.tile([P, 65], F32, tag="psA", name="qx2", space="PSUM")
            nc.tensor.matmul(qx2[:], lhsT=mt_t, rhs=qa2_b[:], start=True,
                             stop=True)
            qe2 = sb2.tile([P, 66], BF16, tag="qe2", name="qe2")
            nc.vector.tensor_copy(out=qe2[:, 0:65], in_=qx2[:])
            qk2 = sb2.tile([P, 64], BF16, tag="qk2", name="qk2")
            nc.vector.tensor_tensor(out=qk2[:], in0=qe2[:, 0:64], in1=k2_t,
                                    op=ALU.mult)
            sc2 = sb2.tile([P, 8], F32, tag="sc2", name="sc2")
            nc.vector.tensor_reduce(out=sc2[:, 1:2], in_=qk2[:],
                                    axis=mybir.AxisListType.X, op=ALU.add)
            nc.vector.tensor_tensor(out=sc2[:, 2:3], in0=as2_t,
                                    in1=qe2[:, 64:65], op=ALU.add)
            nc.vector.tensor_scalar(out=sc2[:, 3:4], in0=sc2[:, 2:3],
                                    scalar1=0.2, scalar2=None, op0=ALU.mult)
            nc.vector.tensor_tensor(out=sc2[:, 0:1], in0=sc2[:, 2:3],
                                    in1=sc2[:, 3:4], op=ALU.max)
            w2 = sb2.tile([P, 2], F32, tag="w2", name="w2")
            nc.scalar.activation(w2[:], sc2[:, 0:2], AF.Exp)
            rhs2 = sb2.tile([P, 194], BF16, tag="rhs2", name="rhs2")
            nc.vector.tensor_scalar(out=rhs2[:, 0:64], in0=z2_t,
                                    scalar1=w2[:, 0:1], scalar2=None,
                                    op0=ALU.mult)
            nc.vector.tensor_scalar(out=rhs2[:, 64:128], in0=v2_t,
                                    scalar1=w2[:, 1:2], scalar2=None,
                                    op0=ALU.mult)
            nc.vector.tensor_copy(out=rhs2[:, 128:192], in_=p2_t)
            nc.vector.tensor_copy(out=rhs2[:, 192:194], in_=w2[:])
            nc.tensor.matmul(aggB[:, 0:194], lhsT=m_t, rhs=rhs2[:], start=st,
                             stop=sp)
            t_global += 1

        # ---- block post B ----
        pko = sb2.tile([P, 257], BF16, tag="pko", name="pko")
        nc.sync.dma_start(out=pko[:], in_=packB[rs, 0:257])
        ax2_b = sb2.tile([P, 128], BF16, tag="ax2_b", name="ax2b")
        nc.sync.dma_start(out=ax2_b[:], in_=aux2[rs, :])
        di_b2 = sb2.tile([P, 1], F32, tag="di_b2", name="dib2")
        nc.sync.dma_start(out=di_b2[:], in_=T["deginv"][rs, :])

        agg_s = sb2.tile([P, 194], BF16, tag="agg_s", name="aggs")
        nc.vector.tensor_copy(out=agg_s[:], in_=aggB[:])

        s2 = sb2.tile([P, 8], F32, tag="s2", name="s2")
        nc.vector.tensor_tensor(out=s2[:, 0:1], in0=pko[:, 256:257],
                                in1=qa2_b[:, 64:65], op=ALU.add)
        nc.vector.tensor_scalar(out=s2[:, 1:2], in0=s2[:, 0:1], scalar1=0.2,
                                scalar2=None, op0=ALU.mult)
        nc.vector.tensor_tensor(out=s2[:, 2:3], in0=s2[:, 0:1], in1=s2[:, 1:2],
                                op=ALU.max)
        ws2f = sb2.tile([P, 1], F32, tag="ws2f", name="ws2f")
        nc.scalar.activation(ws2f[:], s2[:, 2:3], AF.Exp)
        agg_d = sb2.tile([P, 2], F32, tag="agg_d", name="aggd")
        nc.vector.tensor_copy(out=agg_d[:], in_=aggB[:, 192:194])
        # numerators / denominators
        nG2 = sb2.tile([P, 64], BF16, tag="nG2", name="nG2")
        nc.vector.tensor_scalar(out=nG2[:], in0=pko[:, 0:64],
                                scalar1=ws2f[:, 0:1], scalar2=None, op0=ALU.mult)
        nc.vector.tensor_tensor(out=nG2[:], in0=nG2[:], in1=agg_s[:, 0:64],
                                op=ALU.add)
        d2 = sb2.tile([P, 8], F32, tag="d2", name="d2")
        nc.vector.tensor_tensor(out=d2[:, 0:1], in0=agg_d[:, 0:1],
                                in1=ws2f[:], op=ALU.add)
        nc.vector.tensor_scalar(out=d2[:, 1:2], in0=agg_d[:, 1:2],
                                scalar1=1e-16, scalar2=None, op0=ALU.max)
        nc.vector.reciprocal(out=d2[:, 2:3], in_=d2[:, 0:1])
        nc.vector.reciprocal(out=d2[:, 3:4], in_=d2[:, 1:2])
        g2 = sb2.tile([P, 64], BF16, tag="g2", name="g2")
        nc.vector.tensor_scalar(out=g2[:], in0=nG2[:], scalar1=d2[:, 2:3],
                                scalar2=None, op0=ALU.mult)
        nc.vector.tensor_tensor(out=g2[:], in0=g2[:], in1=bslice(640, 704),
                                op=ALU.add)
        t2o = sb2.tile([P, 64], BF16, tag="t2o", name="t2o")
        nc.vector.tensor_scalar(out=t2o[:], in0=agg_s[:, 64:128],
                                scalar1=d2[:, 3:4], scalar2=None, op0=ALU.mult)
        nc.vector.tensor_tensor(out=t2o[:], in0=t2o[:], in1=ax2_b[:, 0:64],
                                op=ALU.add)
        sg2 = sb2.tile([P, 64], BF16, tag="sg2", name="sg2")
        nc.vector.tensor_scalar(out=sg2[:], in0=agg_s[:, 128:192],
                                scalar1=di_b2[:, 0:1], scalar2=None,
                                op0=ALU.mult)
        nc.vector.tensor_tensor(out=sg2[:], in0=sg2[:], in1=bslice(576, 640),
                                op=ALU.add)
        nc.vector.tensor_tensor(out=sg2[:], in0=sg2[:], in1=ax2_b[:, 64:128],
                                op=ALU.add)
        h2 = sb2.tile([P, 64], BF16, tag="h2", name="h2")
        nc.vector.tensor_scalar(out=h2[:], in0=sg2[:], scalar1=0.0,
                                scalar2=None, op0=ALU.max)

        # transposes + classifier matmuls
        tpb = ps.tile([64, 3 * P], BF16, tag="tp", name="tpb", space="PSUM")
        nc.tensor.transpose(out=tpb[:, 0:P], in_=g2[:], identity=ident[:])
        nc.tensor.transpose(out=tpb[:, P:2 * P], in_=t2o[:], identity=ident[:])
        nc.tensor.transpose(out=tpb[:, 2 * P:3 * P], in_=h2[:], identity=ident[:])
        tb_s = sb2.tile([64, 3 * P], BF16, tag="tb_s", name="tbs")
        nc.vector.tensor_copy(out=tb_s[:], in_=tpb[:])
        pcls = ps.tile([P, 8], F32, tag="aggT", name="pcls", space="PSUM")
        nc.tensor.matmul(pcls[:, 0:2], lhsT=tb_s[:, 0:P], rhs=wcl[:, 0:2],
                         start=True, stop=True)
        nc.tensor.matmul(pcls[:, 2:4], lhsT=tb_s[:, P:2 * P], rhs=wcl[:, 2:4],
                         start=True, stop=True)
        nc.tensor.matmul(pcls[:, 4:8], lhsT=tb_s[:, 2 * P:3 * P], rhs=ws3[:],
                         start=True, stop=True)
        fin = sb2.tile([P, 8], F32, tag="fin", name="fin")
        nc.vector.tensor_copy(out=fin[:], in_=pcls[:])
        # gat_out|trans_out (+cls biases), r3; p3 table
        go = sb2.tile([P, 8], F32, tag="go", name="go")
        nc.vector.tensor_tensor(out=go[:, 0:2], in0=fin[:, 0:2],
                                in1=bslice(706, 708), op=ALU.add)
        nc.vector.tensor_tensor(out=go[:, 2:4], in0=fin[:, 2:4],
                                in1=bslice(708, 710), op=ALU.add)
        nc.vector.tensor_copy(out=go[:, 4:6], in_=fin[:, 6:8])  # r3
        nc.sync.dma_start(out=gto[rs, :], in_=go[:])
        p3s = sb2.tile([P, 64], F32, tag="p3s", name="p3s")
        nc.vector.tensor_copy(out=p3s[:, 0:2], in_=fin[:, 4:6])
        nc.sync.dma_start(out=p3t[rs, :], in_=p3s[:])

    # =========== exchange C ===========
    for r0 in range(0, u8, 1024):
        r1 = min(r0 + 1024, u8)
        k = r1 - r0
        pgc = sb2.tile([P, 8, 64], F32, tag="pgc", name="pgc")
        nc.gpsimd.dma_gather(
            out_ap=pgc[:, 0:k // P, :], in_ap=p3t[:, :],
            idxs_ap=cidx_sb[:, r0 // 16:r0 // 16 + k // 16],
            num_idxs=k, num_idxs_reg=k, elem_size=64)
        nc.sync.dma_start(
            out=contribC[r0:r1, :].rearrange("(t p) e -> p t e", p=P),
            in_=pgc[:, 0:k // P, :])
    nc.gpsimd.collective_compute(
        "AllToAll", ALU.bypass, replica_groups=[list(range(NCORES))],
        ins=[contribC[:]], outs=[p3H[:]])

    # =========== pass C: SAGE L3 edge loop + final ===========
    getC = chunk_loader(p3H, 64, "C", 8, gidx_sb, nt, dt=F32, need_mt=False)
    w0, w1, w2c = float(wts[0]), float(wts[1]), float(wts[2])
    t_global = 0
    for b in range(nblk):
        rs = slice(b * P, (b + 1) * P)
        aggC = ps.tile([P, 2], F32, tag="aggG", name="aggC", space="PSUM")
        ntb = tb[b]
        for j in range(ntb):
            g_t, m_t, _ = getC(t_global)  # f32 gather!
            st, sp = (j == 0), (j == ntb - 1)
            g3b = sb2.tile([P, 2], BF16, tag="g3b", name="g3b")
            nc.vector.tensor_copy(out=g3b[:], in_=g_t[0:P, 0:2])
            nc.tensor.matmul(aggC[:], lhsT=m_t, rhs=g3b[:], start=st, stop=sp)
            t_global += 1
        go_b = sb2.tile([P, 8], F32, tag="go_b", name="gob")
        nc.sync.dma_start(out=go_b[:], in_=gto[rs, :])
        di_b3 = sb2.tile([P, 1], F32, tag="di_b3", name="dib3")
        nc.sync.dma_start(out=di_b3[:], in_=T["deginv"][rs, :])
        fo = sb2.tile([P, 8], F32, tag="fo", name="fo")
        nc.vector.tensor_scalar(out=fo[:, 0:2], in0=aggC[:],
                                scalar1=di_b3[:, 0:1], scalar2=None,
                                op0=ALU.mult)
        nc.vector.tensor_tensor(out=fo[:, 0:2], in0=fo[:, 0:2],
                                in1=bslice(704, 706), op=ALU.add)
        nc.vector.tensor_tensor(out=fo[:, 0:2], in0=fo[:, 0:2],
                                in1=go_b[:, 4:6], op=ALU.add)  # + h2@Wr3
        nc.vector.tensor_scalar(out=fo[:, 2:4], in0=fo[:, 0:2], scalar1=w0,
                                scalar2=None, op0=ALU.mult)
        nc.vector.tensor_scalar(out=fo[:, 4:6], in0=go_b[:, 0:2], scalar1=w1,
                                scalar2=None, op0=ALU.mult)
        nc.vector.tensor_scalar(out=fo[:, 6:8], in0=go_b[:, 2:4], scalar1=w2c,
                                scalar2=None, op0=ALU.mult)
        nc.vector.tensor_tensor(out=fo[:, 2:4], in0=fo[:, 2:4], in1=fo[:, 4:6],
                                op=ALU.add)
        nc.vector.tensor_tensor(out=fo[:, 2:4], in0=fo[:, 2:4], in1=fo[:, 6:8],
                                op=ALU.add)
        lim = min(P, pl.nb - b * P)
        nc.sync.dma_start(out=T["out"][b * P:b * P + lim, :],
                          in_=fo[0:lim, 2:4])


DEBUG = False
LAST_EXEC_NS = None
LAST_RES = None
_CACHE = {}


def kernel(x, edge_index, params):
    x = np.asarray(x)
    ei = np.asarray(edge_index)
    n = x.shape[0]
    nb = n // NCORES
    src, dst = ei[0].astype(np.int64), ei[1].astype(np.int64)
    pl, cores = make_plan(src, dst, n, nb)

    ew = np.asarray(params["ens_w"], np.float64)
    ew = np.exp(ew - ew.max()); wts = ew / ew.sum()

    GA, GB = 4, 8
    maps = build_inputs(x, params, pl, cores, GA, GB)
    key = (n, ei.shape[1], pl.nt, pl.u8, tuple(pl.tb), tuple(np.round(wts, 12)))
    if key not in _CACHE:
        _CACHE[key] = build_program(pl, wts, GA, GB, n_in=x.shape[1])
    nc = _CACHE[key]
    import os
    trace = bool(int(os.environ.get("KERNEL_TRACE", "0")))
    res = bass_utils.run_bass_kernel_spmd(nc, maps, core_ids=list(range(NCORES)),
                                          trace=trace)
    global LAST_EXEC_NS, LAST_RES
    LAST_EXEC_NS = res.exec_time_ns
    LAST_RES = res
    outs = [res.results[c]["out"] for c in range(NCORES)]
    full = np.concatenate(outs, axis=0).astype(np.float32)
    if DEBUG:
        kernel.dbg = res.results
    return full


if __name__ == "__main__":
    pass
